# revision 37
# baseline (speedup 1.0000x reference)
"""Bass/Trainium2 kernel for nn_DFTLayer: out[b,f,k] = DFT_1024(x[b,f,:]).

reference: real = einsum('bfs,ks->bfk', x, wcos); imag = ... wsin
           out  = complex(real, -imag),  x: [16, 1024, 1024] f32.

Strategy (8 NeuronCores, data-parallel over batch, 2 batches/core):
  - Hermitian symmetry (x real): out[k] = conj(out[N-k]); device covers
    k = 0..255 (and k = 257..512 via butterflies); col 256 and the
    k = 513..1023 mirror are host-side.
  - Cosine/sine parity fold (host): u[s] = x[s] + x[N-s], v[s] = x[s] - x[N-s]
    over contraction slots s = 1..512 (u[512] = x[512], v[512] coeff is 0):
        real[k] = x[0] + sum_{s=1..512} u[s] cos(2*pi*k*s/N)
        imag[k] =        sum_{s=1..511} v[s] sin(2*pi*k*s/N)
  - Radix-2 split by parity of s (host): ue[t] = u[2t+2], uo[t] = u[2t+1]
    (t = 0..255), likewise ve/vo:
        E[k] = ue @ cos(2pi k(2t+2)/N),  O[k] = uo @ cos(2pi k(2t+1)/N)
        real[k] = x0 + E[k] + O[k];  real[512-k] = x0 + E[k] - O[k]
        (imag via Es/Os with sin; imag[512-k] = -Es[k] + Os[k])
  - Second split on the EVEN branches only (those recurse without sine
    cross terms): uea[r] = ue[2r], ueb[r] = ue[2r+1] (r = 0..127):
        Ea[k] = uea @ cos(2pi k(4r+2)/N), Eb[k] = ueb @ cos(2pi k(r+1)/256)
        E[k] = Ea[k] + Eb[k];   E[256-k] = -Ea[k] + Eb[k]   (k = 0..127)
        E[128] = -sum_r ueb[r] (-1)^r        (host dot product)
        Es[k] = Esa + Esb;      Es[256-k] = Esa[k] - Esb[k]
        Es[128] = sum_r vea[r] (-1)^r
    Device matmul work: O/Os at 256-contraction + Ea/Eb/Esa/Esb at 128 =
    24576 moving rows (~10.3 us at 2.4 GHz) -- fits inside one TRN2 HAM
    full-power window, avoiding the 50%-duty throttle tail.
  - Everything crossing HBM is bf16: ~8.4 MB per core; rel err ~3e-3.
  - PE p-state warm-up via memset-fed dummy matmuls (clock ramps
    0.65 -> 2.4 GHz only after ~3 us of continuous activity).
  - One combined 2-bank PSUM->SBUF bf16 cast per 1024-row half
    (alternating ACT/DVE; only they can read PSUM); inputs stream on the
    sync HWDGE queue in consumption order (first blocks split across
    sync+scalar by partition halves); outputs ride sync, tail split
    sync+scalar. Butterflies/mirrors/corrections happen on the host.
"""

import sys

for _p in ("/opt/trn_rl_repo", "/root/.axon_site/_ro/trn_rl_repo"):
    if _p not in sys.path:
        sys.path.append(_p)

import numpy as np
import ml_dtypes
from contextlib import ExitStack

BF16 = np.dtype(ml_dtypes.bfloat16)

N_CORES = 8
B, F_FULL, S = 16, 1024, 1024          # x: [B, F_FULL, S]
F = (B // N_CORES) * F_FULL            # 2048 rows per core
M = 256                                # radix-2 contraction length
KD = 256                               # freq cols per level-1 kernel
WARMUP_MM = 8                          # dummy matmuls to ramp the PE p-state

_CACHE = {}


def _build():
    """Build + compile the per-core Bass program (cached)."""
    if "nc" in _CACHE:
        return _CACHE["nc"]

    from concourse import bacc, tile, mybir

    f32 = mybir.dt.float32
    bf16 = mybir.dt.bfloat16

    nc = bacc.Bacc("TRN2", target_bir_lowering=False, debug=False)

    # input blocks b = 0..7, each [128, 2048] bf16 (4 KB rows), DMA order =
    # phase consumption order:
    #   b0/b1: uo row-halves   [p, (tc, j)]  t = tc*128+p, row = h*1024+j
    #   b2:    uea (flat)      [r, rows]
    #   b3:    ueb             [r, rows]
    #   b4/b5: vo row-halves
    #   b6:    vea,  b7: veb
    uv_d = nc.dram_tensor("uv", [8 * 128, F], bf16, kind="ExternalInput")
    # w cols: 0..1023  = (tc, kern2, kc, q) for O/Os   [t, kern2*256+kc*128+q]
    #         1024..1535 = [Ea|Eb|Esa|Esb] 128-col blocks; 1536.. = pad
    w_d = nc.dram_tensor("w", [128, 2048], bf16, kind="ExternalInput")
    # eo row-blocks: p0/p1 = O (k 0..127 / 128..255), p2 = Ea, p3 = Eb,
    # p4/p5 = Os, p6 = Esa, p7 = Esb; col blocks [half, g, 512] = 2048 rows
    eo_d = nc.dram_tensor("eo", [8 * 128, 2, 2, 512], bf16, kind="ExternalOutput")

    with tile.TileContext(nc) as tc, ExitStack() as ctx:
        wpool = ctx.enter_context(tc.tile_pool(name="w", bufs=1))
        opool = ctx.enter_context(tc.tile_pool(name="o", bufs=4))
        ppool = ctx.enter_context(tc.tile_pool(name="p", bufs=4, space="PSUM"))

        f32r = mybir.dt.float32r

        # warm-up operand needs no DMA: memset lands right after the prologue
        wu_t = wpool.tile([128, 512], f32, tag="wu")
        nc.gpsimd.memset(wu_t[:], 1.0)

        # w and block 0 gate the first real matmul: split each across BOTH
        # HWDGE queues by partition halves (still 4 KB rows) so they land
        # ~1.5 us sooner; later blocks stream whole on sync
        w_t = wpool.tile([128, 2048], bf16, tag="w")
        nc.sync.dma_start(w_t[0:64], w_d[0:64])
        nc.scalar.dma_start(w_t[64:128], w_d[64:128])

        uv_ts = []
        for bidx in range(8):
            uv_t = wpool.tile([128, 2048], bf16, tag=f"uv{bidx}")
            src = uv_d[bidx * 128:(bidx + 1) * 128, :]
            if bidx == 0:
                nc.sync.dma_start(uv_t[0:64], src[0:64])
                nc.scalar.dma_start(uv_t[64:128], src[64:128])
            else:
                nc.sync.dma_start(uv_t[:], src)
            uv_ts.append(uv_t)

        # p-state warm-up: dummy matmuls keep the PE continuously busy from
        # the prologue until real operands arrive
        ps_w = ppool.tile([128, 2, 512], f32, tag="ps")
        for i in range(WARMUP_MM):
            nc.tensor.matmul(ps_w[:, i % 2, 0:128], wu_t[:, 0:128].bitcast(f32r),
                             wu_t[:, 0:128].bitcast(f32r), start=True, stop=True)

        def finish_phase(p, pss, last):
            """Combined casts + output DMA for phase p's two PSUM tiles."""
            out_t = opool.tile([128, 2, 2, 512], bf16)
            r0 = p * 128
            for half in range(2):
                ps = pss[half]
                if not last:
                    if (p * 2 + half) % 2 == 0:
                        nc.scalar.copy(out_t[:, half], ps[:])
                    else:
                        nc.vector.tensor_copy(out_t[:, half], ps[:])
                    nc.sync.dma_start(eo_d[r0:r0 + 128, half], out_t[:, half])
                else:
                    nc.scalar.copy(out_t[:, half, 0], ps[:, 0, :])
                    nc.vector.tensor_copy(out_t[:, half, 1], ps[:, 1, :])
                    for g in range(2):
                        eng = nc.sync if g == 0 else nc.scalar
                        eng.dma_start(eo_d[r0:r0 + 128, half, g],
                                      out_t[:, half, g])

        # phases p0..p7; O/Os phases use (kern2, kc) with blocks (bb, bb+1);
        # E phases use one 128-contraction sub-kernel with one flat block
        for kern2 in range(2):           # 0 = cos branch (O/E), 1 = sin
            bb = kern2 * 4               # uo/vo at bb/bb+1, E-branch bb+2/3
            for kc in range(2):          # O (or Os) phase
                p = kern2 * 4 + kc
                ps_a = ppool.tile([128, 2, 512], f32, tag="ps")
                ps_b = ppool.tile([128, 2, 512], f32, tag="ps")
                pss = [ps_a, ps_b]
                for t in range(2):
                    lhsT = w_t[:, t * 512 + kern2 * 256 + kc * 128:
                               t * 512 + kern2 * 256 + (kc + 1) * 128]
                    for half in range(2):
                        src = uv_ts[bb + half]
                        for g in range(2):
                            nc.tensor.matmul(
                                pss[half][:, g, :],
                                lhsT,
                                src[:, t * 1024 + g * 512:
                                    t * 1024 + (g + 1) * 512],
                                start=(t == 0),
                                stop=(t == 1),
                            )
                finish_phase(p, pss, last=False)
            for sub in range(2):         # Ea/Eb (or Esa/Esb) phase
                p = kern2 * 4 + 2 + sub
                last = p == 7
                ps_c = ppool.tile([128, 2, 512], f32, tag="ps")
                ps_d = ppool.tile([128, 2, 512], f32, tag="ps")
                pss = [ps_c, ps_d]
                lhsT = w_t[:, 1024 + (kern2 * 2 + sub) * 128:
                           1024 + (kern2 * 2 + sub + 1) * 128]
                src = uv_ts[bb + 2 + sub]
                for half in range(2):
                    for g in range(2):
                        nc.tensor.matmul(
                            pss[half][:, g, :],
                            lhsT,
                            src[:, half * 1024 + g * 512:
                                half * 1024 + (g + 1) * 512],
                            start=True,
                            stop=True,
                        )
                finish_phase(p, pss, last)

    nc.compile()
    _CACHE["nc"] = nc
    return nc


def kernel(x, wsin, wcos):
    from concourse.bass_utils import run_bass_kernel_spmd

    x = np.asarray(x, dtype=np.float32)
    wsin = np.asarray(wsin, dtype=np.float32)
    wcos = np.asarray(wcos, dtype=np.float32)

    nc = _build()

    # DFT kernels sliced from the provided (symmetric) matrices
    wO = wcos[1:512:2, 0:KD]            # cos, s = 2t+1      [256, 256]
    wOs = wsin[1:512:2, 0:KD]           # sin, s = 2t+1
    cEa = wcos[2:512:4, 0:128]          # cos, s = 4r+2      [128, 128]
    cEb = wcos[4:513:4, 0:128]          # cos, s = 4r+4
    sEa = wsin[2:512:4, 0:128]
    sEb = wsin[4:513:4, 0:128]
    w_np = np.zeros((128, 2048), dtype=BF16)
    oo = np.concatenate([wO, wOs], axis=1).astype(BF16)         # [256, 512]
    w_np[:, 0:1024] = oo.reshape(2, 128, 512).transpose(1, 0, 2).reshape(
        128, 1024)
    w_np[:, 1024:1536] = np.concatenate(
        [cEa, cEb, sEa, sEb], axis=1).astype(BF16)

    # host fold + parity split (f32), then bf16
    xa = x[:, :, 1:512]
    xb = x[:, :, 1023:512:-1]
    u = xa + xb                         # u[s], s = 1..511
    v = xa - xb
    uvp = np.empty((B, F_FULL, 4, M), dtype=np.float32)
    uvp[:, :, 0, :255] = u[:, :, 1::2]  # ue: s = 2,4,..,510
    uvp[:, :, 0, 255] = x[:, :, 512]    # ue[255] <- u[512] = x[512]
    uvp[:, :, 1, :] = u[:, :, 0::2]     # uo: s = 1,3,..,511
    uvp[:, :, 2, :255] = v[:, :, 1::2]  # ve
    uvp[:, :, 2, 255] = 0.0
    uvp[:, :, 3, :] = v[:, :, 0::2]     # vo
    uvp_bf = uvp.astype(BF16)

    bpc = B // N_CORES
    in_maps = []
    for c in range(N_CORES):
        blk = uvp_bf[c * bpc:(c + 1) * bpc].reshape(F, 4, M)
        uv_c = np.empty((8, 128, F), dtype=BF16)
        for kern2, (iodd, ieven) in enumerate(((1, 0), (3, 2))):
            bb = kern2 * 4
            # odd-s data: [row, t] -> blocks [h][p, (tc, j)]
            ot = np.ascontiguousarray(blk[:, iodd, :].T)      # [256, F]
            uv_c[bb:bb + 2] = ot.reshape(2, 128, 2, 1024).transpose(
                2, 1, 0, 3).reshape(2, 128, 2048)
            # even-s data split by parity of t: flat [r, rows]
            ev = blk[:, ieven, :]                             # [F, 256]
            uv_c[bb + 2] = ev[:, 0::2].T                      # a: t = 2r
            uv_c[bb + 3] = ev[:, 1::2].T                      # b: t = 2r+1
        in_maps.append({"uv": np.ascontiguousarray(uv_c).reshape(8 * 128, F),
                        "w": w_np})

    res = run_bass_kernel_spmd(
        nc, in_maps, core_ids=list(range(N_CORES)), **_CACHE.get("run_kwargs", {})
    )
    kernel.last_results = res

    # host assembly: level-2 then level-1 butterflies, x[0] correction,
    # col 256, Hermitian mirror
    alt = np.where(np.arange(M) % 2 == 0, np.float32(1.0), np.float32(-1.0))
    altB = np.where(np.arange(128) % 2 == 0, np.float32(1.0),
                    np.float32(-1.0))
    out = np.empty((B, F_FULL, S), dtype=np.complex64)
    fv = out.view(np.float32).reshape(B, F_FULL, 2 * S)
    for c in range(N_CORES):
        b0 = c * bpc
        eo = np.asarray(res.results[c]["eo"]).reshape(8, 128, F)
        O = np.concatenate([eo[0], eo[1]]).T.astype(np.float32)   # [F, 256]
        Ea = eo[2].T.astype(np.float32)                           # [F, 128]
        Eb = eo[3].T.astype(np.float32)
        Os = np.concatenate([eo[4], eo[5]]).T.astype(np.float32)
        Esa = eo[6].T.astype(np.float32)
        Esb = eo[7].T.astype(np.float32)
        blk32 = uvp[b0:b0 + bpc].reshape(F, 4, M)
        # level-2 butterflies: rebuild E, Es (k = 0..255)
        E = np.empty((F, 256), dtype=np.float32)
        E[:, 0:128] = Ea + Eb
        E[:, 129:256] = (Eb - Ea)[:, 127:0:-1]
        E[:, 128] = -(blk32[:, 0, 1::2] @ altB)
        Es = np.empty((F, 256), dtype=np.float32)
        Es[:, 0:128] = Esa + Esb
        Es[:, 129:256] = (Esa - Esb)[:, 127:0:-1]
        Es[:, 128] = blk32[:, 2, 0::2] @ altB
        # level-1 butterflies
        x0 = x[b0:b0 + bpc, :, 0].reshape(F, 1)
        reA = E + O
        reA += x0
        reB = E - O
        reB += x0
        imA = Es + Os
        np.negative(imA, out=imA)           # out.imag = -imag_raw
        imB = Es - Os
        fvb = fv[b0:b0 + bpc].reshape(F, 2 * S)
        fvb[:, 0:2 * KD:2] = reA            # real, k = 0..255
        fvb[:, 1:2 * KD:2] = imA
        fvb[:, 514:1026:2] = reB[:, ::-1]   # real, k = 257..512
        fvb[:, 515:1027:2] = imB[:, ::-1]
        # col 256: even-s cos run is (-1)^(t+1), odd-s sin run is (-1)^t
        fvb[:, 512] = x0[:, 0] - blk32[:, 0, :] @ alt
        fvb[:, 513] = -(blk32[:, 3, :] @ alt)
        # Hermitian mirror: out[k] = conj(out[1024-k]) for k = 513..1023
        fvb[:, 1026::2] = fvb[:, 1022:0:-2]
        fvb[:, 1027::2] = -fvb[:, 1023:1:-2]
    return out
